# revision 9
# baseline (speedup 1.0000x reference)
"""GATv2 message passing on 8 Trainium2 NeuronCores (Bass/Tile).

Strategy (edge-parallel by receiver ownership, host-pregathered streams):
  - Nodes split into 8 contiguous ranges of 6250; core c owns range c and all
    edges whose receiver falls in it (no cross-core reduction).
  - The HOST pre-gathers raw endpoint features into receiver-window-sorted
    edge order (pure indexing, same prep class as the baseline's sort) and
    ships bf16 streams: sT/rT [fin, e] feature-major per 128-edge chunk, plus
    the per-chunk one-hot scatter matrices oh[e, slot].  The device does only
    SEQUENTIAL DMA -- no SWDGE gathers (the original 2.4ms bottleneck).
  - Per chunk the PE accumulates x = W_s(sent) + W_r(recv) in PSUM (2
    matmuls, bf16).  The separate "edges" tensor is never materialized:
    using edges = x - tabr[slot] (tabr = W_r projection of the core's own
    nodes, computed once in a prologue), the attention-weighted mean obeys
        out[slot] = segsum(w*x)/segsum(w) - tabr[slot]
    which moves the correction entirely to the [6250,128]-sized epilogue.
  - mish/softmax chain (exact, one act table: exp/square/ln/identity):
      u = exp(x); q = (u+1)^2; v = ln(q+1); g = exp(-v + ln2) = 2/(q+1)
      hm_neg = (g-1) * (x*a) = -mish(x)*a   per feature
      lgt_neg = sum_d hm_neg;  w = exp(-lgt_neg + attn_b)
    msgw = [x*w | w] ([e,136]) so ONE one-hot matmul per chunk scatters both
    the numerator and the softmax denominator into the window accumulator.
  - out = (num - tabr*den)/den per window; host reassembles the 8 slices.
"""

import os
import sys

for _p in ("/opt/trn_rl_repo", "/root/.axon_site/_ro/trn_rl_repo"):
    if os.path.isdir(_p) and _p not in sys.path:
        sys.path.insert(0, _p)

import numpy as np
import ml_dtypes

import concourse.bass as bass
import concourse.bacc as bacc
import concourse.tile as tile
from concourse import mybir
from concourse import bass_utils

F32 = mybir.dt.float32
BF16 = mybir.dt.bfloat16

N_NODES = 50000
N_EDGES = 800000
F = 128            # feature dim
H = 8              # heads
D = 16             # head dim
NCORE = 8
NPC = N_NODES // NCORE          # 6250 nodes per core
WIN = 128                       # nodes per scatter window
NWIN = (NPC + WIN - 1) // WIN   # 49 windows per core
NLP = NWIN * WIN                # local nodes padded (6272)
CHUNK = 128                     # edges per matmul chunk
GRP = 8                         # chunks per elementwise group
BLK = 32                        # chunks per DMA block (4096 edges)
MW = F + H                      # msg+weight columns per chunk (136)
LN2 = float(np.log(2.0))

_prog_cache = {}


def _build_program(cpw, nblk, attn_bias, with_xbias):
    """SPMD Bass program: cpw chunks per window, nblk DMA blocks."""
    n_real = NWIN * cpw

    nc = bacc.Bacc("TRN2", target_bir_lowering=False, debug=False,
                   enable_asserts=False, num_devices=NCORE)

    inp = {}
    def dram_in(name, shape, dt=F32):
        inp[name] = nc.dram_tensor(name, list(shape), dt, kind="ExternalInput").ap()
        return inp[name]

    ws_in = dram_in("ws", (F, F), BF16)            # [fin, fout]
    wr_in = dram_in("wr", (F, F), BF16)
    attn_in = dram_in("attn", (128, F))            # attn replicated, f32
    nloc_in = dram_in("nlocT", (F, NLP), BF16)     # local nodes feature-major
    sT_in = dram_in("sT", (nblk, 128, BLK * CHUNK), BF16)
    rT_in = dram_in("rT", (nblk, 128, BLK * CHUNK), BF16)
    oh_in = dram_in("oh", (nblk, 128, BLK * CHUNK), BF16)
    if with_xbias:
        xbias_in = dram_in("xbias", (128, F))      # (Ws_b+Wr_b) replicated
        wrb_in = dram_in("wrb", (128, F))          # Wr_b replicated
    out_d = nc.dram_tensor("out_d", [NLP, F], F32, kind="ExternalOutput").ap()

    # const APs for activation biases
    for val in {float(attn_bias), LN2}:
        if (F32, val) not in nc.const_aps.aps:
            t = nc.alloc_sbuf_tensor(f"const-{val}", [128, 1], F32)
            nc.gpsimd.memset(t.ap(), val)
            nc.const_aps.aps[(F32, val)] = t.ap()
    nc.all_engine_barrier()
    ab = float(attn_bias)

    def chunk_info(g):
        if g >= n_real:
            return (None, None)
        return divmod(g, cpw)

    with tile.TileContext(nc) as tc:
        with tc.tile_pool(name="const", bufs=1) as cpool, \
             tc.tile_pool(name="stream", bufs=2) as stpool, \
             tc.tile_pool(name="work", bufs=2) as wpool, \
             tc.tile_pool(name="accp", bufs=1) as apool, \
             tc.tile_pool(name="psX", bufs=2, space="PSUM") as psX, \
             tc.tile_pool(name="psA", bufs=2, space="PSUM") as psA:
            ws_t = cpool.tile([F, F], BF16)
            wr_t = cpool.tile([F, F], BF16)
            attn_t = cpool.tile([128, F], F32)
            nloc_t = cpool.tile([F, NLP], BF16)
            nc.sync.dma_start(out=ws_t[:], in_=ws_in[:])
            nc.sync.dma_start(out=wr_t[:], in_=wr_in[:])
            nc.sync.dma_start(out=attn_t[:], in_=attn_in[:])
            nc.sync.dma_start(out=nloc_t[:], in_=nloc_in[:])
            if with_xbias:
                xbias_t = cpool.tile([128, F], F32)
                wrb_t = cpool.tile([128, F], F32)
                nc.sync.dma_start(out=xbias_t[:], in_=xbias_in[:])
                nc.sync.dma_start(out=wrb_t[:], in_=wrb_in[:])

            acc = apool.tile([128, NWIN * MW], F32)
            tabr = apool.tile([128, NWIN * F], F32)

            # ---- prologue: tabr[slot] = W_r(local nodes) (+Wr_b) ----
            for w in range(NWIN):
                ps_t = psA.tile([128, F], F32, space="PSUM", tag="ptab")
                nc.tensor.matmul(ps_t[:],
                                 lhsT=nloc_t[:, w * WIN:(w + 1) * WIN],
                                 rhs=wr_t[:], start=True, stop=True,
                                 skip_group_check=True)
                nc.scalar.copy(tabr[:, w * F:(w + 1) * F], ps_t[:])
            if with_xbias:
                nc.vector.tensor_tensor(
                    tabr[:].rearrange("p (w f) -> p w f", f=F),
                    tabr[:].rearrange("p (w f) -> p w f", f=F),
                    wrb_t[:].unsqueeze(1).to_broadcast([128, NWIN, F]),
                    op=mybir.AluOpType.add)

            # ---- edge stream ----
            agg_ps = None
            for b in range(nblk):
                sT_t = stpool.tile([128, BLK * CHUNK], BF16, tag="sT")
                rT_t = stpool.tile([128, BLK * CHUNK], BF16, tag="rT")
                oh_t = stpool.tile([128, BLK * CHUNK], BF16, tag="oh")
                nc.sync.dma_start(out=sT_t[:], in_=sT_in[b])
                nc.sync.dma_start(out=rT_t[:], in_=rT_in[b])
                nc.sync.dma_start(out=oh_t[:], in_=oh_in[b])

                for g0 in range(0, BLK, GRP):
                    ps_x = psX.tile([128, GRP, F], F32, space="PSUM", tag="psx")
                    for c in range(GRP):
                        cc = g0 + c
                        sl = sT_t[:, cc * CHUNK:(cc + 1) * CHUNK]
                        rl = rT_t[:, cc * CHUNK:(cc + 1) * CHUNK]
                        nc.tensor.matmul(ps_x[:, c, :], lhsT=sl, rhs=ws_t[:],
                                         start=True, stop=False,
                                         skip_group_check=True)
                        nc.tensor.matmul(ps_x[:, c, :], lhsT=rl, rhs=wr_t[:],
                                         start=False, stop=True,
                                         skip_group_check=True)

                    if with_xbias:
                        x_sb = wpool.tile([128, GRP, F], F32, tag="x")
                        nc.vector.tensor_tensor(
                            x_sb[:], ps_x[:],
                            xbias_t[:].unsqueeze(1).to_broadcast([128, GRP, F]),
                            op=mybir.AluOpType.add)
                        x = x_sb[:]
                    else:
                        x = ps_x[:]

                    # xa = x * attn                 [DVE]
                    xa = wpool.tile([128, GRP, F], F32, tag="xa")
                    nc.vector.tensor_tensor(
                        xa[:], x,
                        attn_t[:].unsqueeze(1).to_broadcast([128, GRP, F]),
                        op=mybir.AluOpType.mult)

                    # mish chain                    [Act]
                    u = wpool.tile([128, GRP, F], F32, tag="u")
                    nc.scalar.activation(u[:], x,
                                         mybir.ActivationFunctionType.Exp)
                    q = wpool.tile([128, GRP, F], F32, tag="q")
                    nc.scalar.activation(q[:], u[:],
                                         mybir.ActivationFunctionType.Square,
                                         bias=1.0)
                    v = wpool.tile([128, GRP, F], F32, tag="v")
                    nc.scalar.activation(v[:], q[:],
                                         mybir.ActivationFunctionType.Ln,
                                         bias=1.0)
                    g = wpool.tile([128, GRP, F], F32, tag="g")
                    nc.scalar.activation(g[:], v[:],
                                         mybir.ActivationFunctionType.Exp,
                                         bias=LN2, scale=-1.0)

                    # hm_neg = (g-1)*xa = -mish(x)*attn      [DVE]
                    hm = wpool.tile([128, GRP, F], BF16, tag="hm")
                    nc.vector.scalar_tensor_tensor(
                        hm[:], g[:], 1.0, xa[:],
                        op0=mybir.AluOpType.subtract,
                        op1=mybir.AluOpType.mult)
                    # lgt_neg = sum_d hm_neg        [DVE]
                    lgt = wpool.tile([128, GRP, H], F32, tag="lgt")
                    nc.vector.tensor_reduce(
                        out=lgt[:].unsqueeze(3),
                        in_=hm[:].rearrange("p c (h d) -> p c h d", d=D),
                        op=mybir.AluOpType.add, axis=mybir.AxisListType.X)

                    # w = exp(-lgt_neg + attn_b)    [Act]
                    wv = wpool.tile([128, GRP, H], F32, tag="wv")
                    nc.scalar.activation(wv[:], lgt[:],
                                         mybir.ActivationFunctionType.Exp,
                                         bias=ab, scale=-1.0)
                    msgw = wpool.tile([128, GRP, MW], BF16, tag="msgw")
                    nc.scalar.activation(msgw[:, :, F:MW], lgt[:],
                                         mybir.ActivationFunctionType.Exp,
                                         bias=ab, scale=-1.0)
                    # msg = x * w                   [DVE]
                    nc.vector.tensor_tensor(
                        msgw[:, :, 0:F].rearrange("p c (h d) -> p c h d", d=D),
                        x.rearrange("p c (h d) -> p c h d", d=D),
                        wv[:].unsqueeze(3).to_broadcast([128, GRP, H, D]),
                        op=mybir.AluOpType.mult)

                    # scatter                       [PE]
                    for c in range(GRP):
                        g_ch = b * BLK + g0 + c
                        w_idx, pos = chunk_info(g_ch)
                        if w_idx is None:
                            continue
                        if pos == 0:
                            agg_ps = psA.tile([128, MW], F32, space="PSUM",
                                              tag="agg")
                        cc = g0 + c
                        nc.tensor.matmul(agg_ps[:],
                                         lhsT=oh_t[:, cc * CHUNK:(cc + 1) * CHUNK],
                                         rhs=msgw[:, c, :],
                                         start=(pos == 0),
                                         stop=(pos == cpw - 1),
                                         skip_group_check=True)
                        if pos == cpw - 1:
                            nc.scalar.copy(acc[:, w_idx * MW:(w_idx + 1) * MW],
                                           agg_ps[:])

            # ---- epilogue: out = (num - tabr*den)/den ----
            accv = acc[:].rearrange("p (w k) -> p w k", k=MW)
            den = accv[:, :, F:MW]
            nc.vector.tensor_scalar_add(den, den, 1e-30)
            # num' = num - tabr*den, then * 1/den  (td reused in-place)
            td = apool.tile([128, NWIN * F], F32)
            nc.vector.tensor_tensor(
                td[:].rearrange("p (w h d) -> p w h d", h=H, d=D),
                tabr[:].rearrange("p (w h d) -> p w h d", h=H, d=D),
                den.rearrange("p w (h o) -> p w h o", o=1)
                   .to_broadcast([128, NWIN, H, D]),
                op=mybir.AluOpType.mult)
            nc.vector.tensor_tensor(
                td[:].rearrange("p (w f) -> p w f", f=F),
                accv[:, :, 0:F],
                td[:].rearrange("p (w f) -> p w f", f=F),
                op=mybir.AluOpType.subtract)
            rcp = apool.tile([128, NWIN * H], F32)
            scr = apool.tile([128, NWIN * H], F32)
            nc.vector.reciprocal_approx_accurate(
                rcp[:].rearrange("p (w h) -> p w h", h=H), den, scr[:])
            nc.vector.tensor_tensor(
                td[:].rearrange("p (w h d) -> p w h d", h=H, d=D),
                td[:].rearrange("p (w h d) -> p w h d", h=H, d=D),
                rcp[:].rearrange("p (w h) -> p w h", h=H).unsqueeze(3)
                      .to_broadcast([128, NWIN, H, D]),
                op=mybir.AluOpType.mult)
            nc.sync.dma_start(
                out=out_d[:].rearrange("(w p) f -> p w f", p=128),
                in_=td[:].rearrange("p (w f) -> p w f", f=F))

    nc.compile()
    return nc


def _prep_core(nodes_bf, senders, receivers, core, cpw, nblk):
    """Pre-gather per-core edge streams + one-hot (host-side indexing only)."""
    e_pad = nblk * BLK * CHUNK
    mask = (receivers // NPC) == core
    s = senders[mask].astype(np.int64)
    r = receivers[mask].astype(np.int64)
    rl = r - core * NPC
    win = rl // WIN
    order = np.argsort(win, kind="stable")
    s, r, rl, win = s[order], r[order], rl[order], win[order]

    pos = np.arange(len(win)) - np.searchsorted(win, win)
    slot = win * (cpw * CHUNK) + pos
    assert pos.max(initial=0) < cpw * CHUNK

    sidx = np.zeros(e_pad, np.int64)
    ridx = np.zeros(e_pad, np.int64)
    sidx[slot] = s
    ridx[slot] = r

    sT = nodes_bf[sidx].reshape(nblk, BLK * CHUNK, F).transpose(0, 2, 1).copy()
    rT = nodes_bf[ridx].reshape(nblk, BLK * CHUNK, F).transpose(0, 2, 1).copy()

    # one-hot stream: oh[b, p, cc*128 + j] = (slot-in-window of edge == j)
    oh = np.zeros((nblk * BLK * CHUNK, 128), ml_dtypes.bfloat16)
    oh[slot, (rl - win * WIN)] = 1.0
    oh = oh.reshape(nblk, BLK, CHUNK, 128)      # [b, cc, p, j]
    oh = oh.transpose(0, 2, 1, 3).reshape(nblk, 128, BLK * 128).copy()
    return sT, rT, oh


def kernel(nodes, senders, receivers, Ws_k, Ws_b, Wr_k, Wr_b, attn_k, attn_b):
    nodes = np.asarray(nodes, np.float32)
    senders = np.asarray(senders, np.int32)
    receivers = np.asarray(receivers, np.int32)
    assert nodes.shape == (N_NODES, F) and senders.shape == (N_EDGES,)

    core_of = receivers // NPC
    win = (receivers - core_of * NPC) // WIN
    key = core_of.astype(np.int64) * NWIN + win
    counts = np.bincount(key, minlength=NCORE * NWIN)
    cpw = max(1, int(np.ceil(counts.max() / CHUNK)))
    nblk = (NWIN * cpw + BLK - 1) // BLK

    wsb = np.asarray(Ws_b, np.float32).reshape(F)
    wrb = np.asarray(Wr_b, np.float32).reshape(F)
    ab = float(np.asarray(attn_b, np.float32).ravel()[0])
    with_xbias = bool(np.any(wsb != 0) or np.any(wrb != 0))

    ck = (cpw, nblk, ab, with_xbias)
    if ck not in _prog_cache:
        _prog_cache[ck] = _build_program(*ck)
    nc = _prog_cache[ck]

    nodes_bf = nodes.astype(ml_dtypes.bfloat16)
    ws = np.asarray(Ws_k, np.float32).reshape(F, F).astype(ml_dtypes.bfloat16)
    wr = np.asarray(Wr_k, np.float32).reshape(F, F).astype(ml_dtypes.bfloat16)
    a_flat = np.tile(np.asarray(attn_k, np.float32).ravel(), H)
    attn = np.broadcast_to(a_flat, (128, F)).copy()

    in_maps = []
    for c in range(NCORE):
        sT, rT, oh = _prep_core(nodes_bf, senders, receivers, c, cpw, nblk)
        nlocT = np.zeros((F, NLP), ml_dtypes.bfloat16)
        nlocT[:, :NPC] = nodes_bf[c * NPC:(c + 1) * NPC].T
        im = {"ws": ws, "wr": wr, "attn": attn, "nlocT": nlocT,
              "sT": sT, "rT": rT, "oh": oh}
        if with_xbias:
            im["xbias"] = np.broadcast_to(wsb + wrb, (128, F)).copy()
            im["wrb"] = np.broadcast_to(wrb, (128, F)).copy()
        in_maps.append(im)

    trace = bool(int(os.environ.get("GAT_TRACE", "0")))
    res = bass_utils.run_bass_kernel_spmd(nc, in_maps,
                                          core_ids=list(range(NCORE)),
                                          trace=trace)
    if trace:
        kernel.last_profile = res
    out = np.empty((N_NODES, F), np.float32)
    for c in range(NCORE):
        out[c * NPC:(c + 1) * NPC] = np.asarray(res.results[c]["out_d"])[:NPC]
    return out


# revision 17
# speedup vs baseline: 1.2226x; 1.2226x over previous
"""GATv2 message passing on 8 Trainium2 NeuronCores (Bass/Tile).

Strategy (edge-parallel by receiver ownership, host-pregathered streams):
  - Nodes split into 8 contiguous ranges of 6250; core c owns range c and all
    edges whose receiver falls in it (no cross-core reduction).
  - The HOST pre-gathers raw endpoint features into receiver-window-sorted
    edge order (pure indexing, same class of prep as the baseline's sort) and
    ships them as bf16 streams laid out feature-major per 128-edge chunk:
    sT[fin, e], rT[fin, e].  The device then does only SEQUENTIAL DMA -- no
    SWDGE gathers at all (the previous bottleneck: 2.4ms of descriptor gen).
  - Per chunk the PE projects both endpoints (lhsT = streamed tile, rhs = Ws
    or Wr in bf16): ps_e = W_s(sent) (the "edges" messages) and ps_x =
    ps_e + W_r(recv) accumulated in PSUM.  mish/logits use the exact
    exp/square/recip chain split across Act/DVE (one act table):
      u = exp(x); w2 = (u+1)^2; w2p1 = w2+1   (Act: exp/square/identity)
      rr = 1/w2p1                             (DVE recip_approx_fast)
      xa2 = x * (2*attn)                      (DVE)
      hm_neg = (rr - 0.5) * xa2 = -mish(x)*attn   (DVE stt)
      lgt_neg = sum_d hm_neg                  (DVE reduce)
      w = exp(-lgt_neg + attn_b)              (Act, scale=-1)
    Messages msg = ps_e * w go into a [e,136] tile whose tail 8 columns hold
    w itself, so ONE one-hot matmul per chunk scatters both the numerator and
    the softmax denominator into the per-window PSUM accumulator.
  - out[n] = segsum(w*msg)/segsum(w), normalized on-device; host reassembles
    the [50000,128] output from the 8 slices.
"""

import os
import sys

for _p in ("/opt/trn_rl_repo", "/root/.axon_site/_ro/trn_rl_repo"):
    if os.path.isdir(_p) and _p not in sys.path:
        sys.path.insert(0, _p)

import numpy as np
import ml_dtypes

import concourse.bass as bass
import concourse.bacc as bacc
import concourse.tile as tile
from concourse import mybir
from concourse import bass_utils

F32 = mybir.dt.float32
BF16 = mybir.dt.bfloat16

N_NODES = 50000
N_EDGES = 800000
F = 128            # feature dim
H = 8              # heads
D = 16             # head dim
NCORE = 8
NPC = N_NODES // NCORE          # 6250 nodes per core
WIN = 128                       # nodes per scatter window
NWIN = (NPC + WIN - 1) // WIN   # 49 windows per core
CHUNK = 128                     # edges per matmul chunk
GRP = 4                         # chunks per elementwise group
BLK = 32                        # chunks per DMA block (4096 edges)
MW = F + H                      # msg+weight columns per chunk (136)

_prog_cache = {}


def _build_program(cpw, nblk, attn_bias, with_xbias, with_wsb):
    """SPMD Bass program: cpw chunks per window, nblk DMA blocks."""
    n_real = NWIN * cpw

    nc = bacc.Bacc("TRN2", target_bir_lowering=False, debug=False,
                   enable_asserts=False, num_devices=NCORE)

    inp = {}
    def dram_in(name, shape, dt=F32):
        inp[name] = nc.dram_tensor(name, list(shape), dt, kind="ExternalInput").ap()
        return inp[name]

    ws_in = dram_in("ws", (F, F), BF16)            # [fin, fout]
    wr_in = dram_in("wr", (F, F), BF16)
    attn2_in = dram_in("attn2", (128, F))          # 2*attn replicated, f32
    sT_in = dram_in("sT", (nblk, 128, BLK * CHUNK), BF16)
    rT_in = dram_in("rT", (nblk, 128, BLK * CHUNK), BF16)
    oh_in = dram_in("oh", (nblk, 128, BLK * CHUNK), BF16)
    if with_xbias:
        xbias_in = dram_in("xbias", (128, F))      # (Ws_b+Wr_b) replicated
    if with_wsb:
        wsb_in = dram_in("wsb", (128, F))          # Ws_b replicated
    out_d = nc.dram_tensor("out_d", [NWIN * WIN, F], F32,
                           kind="ExternalOutput").ap()

    # const AP for the exp bias (activation float biases need registration)
    ab = float(attn_bias)
    if (F32, ab) not in nc.const_aps.aps:
        t = nc.alloc_sbuf_tensor(f"const-ab", [128, 1], F32)
        nc.gpsimd.memset(t.ap(), ab)
        nc.const_aps.aps[(F32, ab)] = t.ap()
        nc.all_engine_barrier()

    def chunk_info(g):
        if g >= n_real:
            return (None, None)
        return divmod(g, cpw)

    with tile.TileContext(nc) as tc:
        with tc.tile_pool(name="const", bufs=1) as cpool, \
             tc.tile_pool(name="stream", bufs=2) as stpool, \
             tc.tile_pool(name="work", bufs=2) as wpool, \
             tc.tile_pool(name="accp", bufs=1) as apool, \
             tc.tile_pool(name="psE", bufs=2, space="PSUM") as psE, \
             tc.tile_pool(name="psR", bufs=2, space="PSUM") as psR, \
             tc.tile_pool(name="psA", bufs=2, space="PSUM") as psA:
            ws_t = cpool.tile([F, F], BF16)
            wr_t = cpool.tile([F, F], BF16)
            attn2_t = cpool.tile([128, F], F32)
            nc.sync.dma_start(out=ws_t[:], in_=ws_in[:])
            nc.sync.dma_start(out=wr_t[:], in_=wr_in[:])
            nc.sync.dma_start(out=attn2_t[:], in_=attn2_in[:])
            if with_xbias:
                xbias_t = cpool.tile([128, F], F32)
                nc.sync.dma_start(out=xbias_t[:], in_=xbias_in[:])
            if with_wsb:
                wsb_t = cpool.tile([128, F], F32)
                nc.sync.dma_start(out=wsb_t[:], in_=wsb_in[:])

            acc = apool.tile([128, NWIN * MW], F32)

            agg_ps = None
            for b in range(nblk):
                sT_t = stpool.tile([128, BLK * CHUNK], BF16, tag="sT")
                rT_t = stpool.tile([128, BLK * CHUNK], BF16, tag="rT")
                oh_t = stpool.tile([128, BLK * CHUNK], BF16, tag="oh")
                nc.sync.dma_start(out=sT_t[:], in_=sT_in[b])
                nc.sync.dma_start(out=rT_t[:], in_=rT_in[b])
                nc.sync.dma_start(out=oh_t[:], in_=oh_in[b])

                for g0 in range(0, BLK, GRP):
                    ps_e = psE.tile([128, GRP, F], F32, space="PSUM", tag="pse")
                    ps_x = psR.tile([128, GRP, F], F32, space="PSUM", tag="psx")
                    for c in range(GRP):
                        cc = g0 + c
                        sl = sT_t[:, cc * CHUNK:(cc + 1) * CHUNK]
                        rl = rT_t[:, cc * CHUNK:(cc + 1) * CHUNK]
                        nc.tensor.matmul(ps_e[:, c, :], lhsT=sl, rhs=ws_t[:],
                                         start=True, stop=True,
                                         skip_group_check=True)
                        nc.tensor.matmul(ps_x[:, c, :], lhsT=sl, rhs=ws_t[:],
                                         start=True, stop=False,
                                         skip_group_check=True)
                        nc.tensor.matmul(ps_x[:, c, :], lhsT=rl, rhs=wr_t[:],
                                         start=False, stop=True,
                                         skip_group_check=True)

                    # x = s_proj + r_proj accumulated on PE; optional bias add
                    if with_xbias:
                        x_sb = wpool.tile([128, GRP, F], F32, tag="x")
                        nc.vector.tensor_tensor(
                            x_sb[:], ps_x[:],
                            xbias_t[:].unsqueeze(1).to_broadcast([128, GRP, F]),
                            op=mybir.AluOpType.add)
                        x = x_sb[:]
                    else:
                        x = ps_x[:]
                    # xa2 = x * 2*attn              [DVE: reads PSUM]
                    xa2 = wpool.tile([128, GRP, F], F32, tag="xa2")
                    nc.vector.tensor_tensor(
                        xa2[:], x,
                        attn2_t[:].unsqueeze(1).to_broadcast([128, GRP, F]),
                        op=mybir.AluOpType.mult)
                    # mish chain                    [Act]
                    u = wpool.tile([128, GRP, F], F32, tag="u")
                    nc.scalar.activation(u[:], x,
                                         mybir.ActivationFunctionType.Exp)
                    w2 = wpool.tile([128, GRP, F], F32, tag="w2")
                    nc.scalar.activation(w2[:], u[:],
                                         mybir.ActivationFunctionType.Square,
                                         bias=1.0)
                    w2p1 = wpool.tile([128, GRP, F], F32, tag="w2p1")
                    nc.scalar.activation(w2p1[:], w2[:],
                                         mybir.ActivationFunctionType.Identity,
                                         bias=1.0)

                    # rr = 1/((u+1)^2+1)            [DVE]
                    rr = wpool.tile([128, GRP, F], F32, tag="rr")
                    nc.vector.reciprocal_approx_fast(rr[:], w2p1[:])
                    # hm_neg = (rr-0.5)*xa2 = -mish(x)*attn  [DVE]
                    hm = wpool.tile([128, GRP, F], BF16, tag="hm")
                    nc.vector.scalar_tensor_tensor(
                        hm[:], rr[:], 0.5, xa2[:],
                        op0=mybir.AluOpType.subtract,
                        op1=mybir.AluOpType.mult)
                    # lgt_neg = sum_d hm_neg        [DVE]
                    lgt = wpool.tile([128, GRP, H], F32, tag="lgt")
                    nc.vector.tensor_reduce(
                        out=lgt[:].unsqueeze(3),
                        in_=hm[:].rearrange("p c (h d) -> p c h d", d=D),
                        op=mybir.AluOpType.add, axis=mybir.AxisListType.X)

                    # w = exp(-lgt_neg + attn_b)    [Act]
                    wv = wpool.tile([128, GRP, H], F32, tag="wv")
                    nc.scalar.activation(wv[:], lgt[:],
                                         mybir.ActivationFunctionType.Exp,
                                         bias=ab, scale=-1.0)
                    msgw = wpool.tile([128, GRP, MW], BF16, tag="msgw")
                    nc.scalar.activation(msgw[:, :, F:MW], lgt[:],
                                         mybir.ActivationFunctionType.Exp,
                                         bias=ab, scale=-1.0)
                    # msg = s_proj * w              [DVE]
                    nc.vector.tensor_tensor(
                        msgw[:, :, 0:F].rearrange("p c (h d) -> p c h d", d=D),
                        ps_e[:].rearrange("p c (h d) -> p c h d", d=D),
                        wv[:].unsqueeze(3).to_broadcast([128, GRP, H, D]),
                        op=mybir.AluOpType.mult)

                    # scatter                       [PE]
                    for c in range(GRP):
                        g_ch = b * BLK + g0 + c
                        w_idx, pos = chunk_info(g_ch)
                        if w_idx is None:
                            continue
                        if pos == 0:
                            agg_ps = psA.tile([128, MW], F32, space="PSUM",
                                              tag="agg")
                        cc = g0 + c
                        nc.tensor.matmul(agg_ps[:],
                                         lhsT=oh_t[:, cc * CHUNK:(cc + 1) * CHUNK],
                                         rhs=msgw[:, c, :],
                                         start=(pos == 0),
                                         stop=(pos == cpw - 1),
                                         skip_group_check=True)
                        if pos == cpw - 1:
                            nc.scalar.copy(acc[:, w_idx * MW:(w_idx + 1) * MW],
                                           agg_ps[:])

            # ---------------- normalize + store ----------------
            accv = acc[:].rearrange("p (w k) -> p w k", k=MW)
            den = accv[:, :, F:MW]
            nc.vector.tensor_scalar_add(den, den, 1e-30)
            rcp = wpool.tile([128, NWIN * H], F32, tag="rcp")
            scr = wpool.tile([128, NWIN * H], F32, tag="scr")
            nc.vector.reciprocal_approx_accurate(
                rcp[:].rearrange("p (w h) -> p w h", h=H), den, scr[:])
            outb = wpool.tile([128, NWIN * F], F32, tag="outb")
            nc.vector.tensor_tensor(
                outb[:].rearrange("p (w h d) -> p w h d", h=H, d=D),
                accv[:, :, 0:F].rearrange("p w (h d) -> p w h d", d=D),
                rcp[:].rearrange("p (w h) -> p w h", h=H).unsqueeze(3)
                      .to_broadcast([128, NWIN, H, D]),
                op=mybir.AluOpType.mult)
            if with_wsb:
                nc.vector.tensor_tensor(
                    outb[:].rearrange("p (w f) -> p w f", f=F),
                    outb[:].rearrange("p (w f) -> p w f", f=F),
                    wsb_t[:].unsqueeze(1).to_broadcast([128, NWIN, F]),
                    op=mybir.AluOpType.add)
            nc.sync.dma_start(
                out=out_d[:].rearrange("(w p) f -> p w f", p=128),
                in_=outb[:].rearrange("p (w f) -> p w f", f=F))

    nc.compile()
    return nc


def _prep_core(nodes_bf, senders, receivers, core, cpw, nblk):
    """Pre-gather the per-core edge streams (host-side indexing only)."""
    e_pad = nblk * BLK * CHUNK
    mask = (receivers // NPC) == core
    s = senders[mask].astype(np.int64)
    r = receivers[mask].astype(np.int64)
    rl = r - core * NPC
    win = rl // WIN
    order = np.argsort(win, kind="stable")
    s, r, rl, win = s[order], r[order], rl[order], win[order]

    # slot of each edge: window base + rank within window
    pos = np.arange(len(win)) - np.searchsorted(win, win)
    slot = win * (cpw * CHUNK) + pos
    assert pos.max(initial=0) < cpw * CHUNK

    sidx = np.zeros(e_pad, np.int64)
    ridx = np.zeros(e_pad, np.int64)
    sidx[slot] = s
    ridx[slot] = r

    # feature-major bf16 streams: [nblk, 128 fin, BLK*CHUNK edges]
    sT = nodes_bf[sidx].reshape(nblk, BLK * CHUNK, F).transpose(0, 2, 1).copy()
    rT = nodes_bf[ridx].reshape(nblk, BLK * CHUNK, F).transpose(0, 2, 1).copy()

    # one-hot stream: oh[b, p, cc*128 + j] = (slot-in-window of edge == j);
    # padding slots stay all-zero so they contribute nothing to the scatter
    oh = np.zeros((e_pad, 128), ml_dtypes.bfloat16)
    oh[slot, (rl - win * WIN)] = 1.0
    oh = oh.reshape(nblk, BLK, CHUNK, 128)      # [b, cc, p, j]
    oh = oh.transpose(0, 2, 1, 3).reshape(nblk, 128, BLK * 128).copy()
    return sT, rT, oh


def kernel(nodes, senders, receivers, Ws_k, Ws_b, Wr_k, Wr_b, attn_k, attn_b):
    nodes = np.asarray(nodes, np.float32)
    senders = np.asarray(senders, np.int32)
    receivers = np.asarray(receivers, np.int32)
    assert nodes.shape == (N_NODES, F) and senders.shape == (N_EDGES,)

    core_of = receivers // NPC
    win = (receivers - core_of * NPC) // WIN
    key = core_of.astype(np.int64) * NWIN + win
    counts = np.bincount(key, minlength=NCORE * NWIN)
    cpw = max(1, int(np.ceil(counts.max() / CHUNK)))
    nblk = (NWIN * cpw + BLK - 1) // BLK

    wsb = np.asarray(Ws_b, np.float32).reshape(F)
    wrb = np.asarray(Wr_b, np.float32).reshape(F)
    ab = float(np.asarray(attn_b, np.float32).ravel()[0])
    with_xbias = bool(np.any(wsb != 0) or np.any(wrb != 0))
    with_wsb = bool(np.any(wsb != 0))

    ck = (cpw, nblk, ab, with_xbias, with_wsb)
    if ck not in _prog_cache:
        _prog_cache[ck] = _build_program(*ck)
    nc = _prog_cache[ck]

    nodes_bf = nodes.astype(ml_dtypes.bfloat16)
    ws = np.asarray(Ws_k, np.float32).reshape(F, F).astype(ml_dtypes.bfloat16)
    wr = np.asarray(Wr_k, np.float32).reshape(F, F).astype(ml_dtypes.bfloat16)
    a_flat = np.tile(np.asarray(attn_k, np.float32).ravel(), H)
    attn2 = np.broadcast_to(2.0 * a_flat, (128, F)).copy()

    in_maps = []
    for c in range(NCORE):
        sT, rT, oh = _prep_core(nodes_bf, senders, receivers, c, cpw, nblk)
        im = {"ws": ws, "wr": wr, "attn2": attn2,
              "sT": sT, "rT": rT, "oh": oh}
        if with_xbias:
            im["xbias"] = np.broadcast_to(wsb + wrb, (128, F)).copy()
        if with_wsb:
            im["wsb"] = np.broadcast_to(wsb, (128, F)).copy()
        in_maps.append(im)

    trace = bool(int(os.environ.get("GAT_TRACE", "0")))
    res = bass_utils.run_bass_kernel_spmd(nc, in_maps,
                                          core_ids=list(range(NCORE)),
                                          trace=trace)
    if trace:
        kernel.last_profile = res
    out = np.empty((N_NODES, F), np.float32)
    for c in range(NCORE):
        out[c * NPC:(c + 1) * NPC] = np.asarray(res.results[c]["out_d"])[:NPC]
    return out


# revision 18
# speedup vs baseline: 1.3858x; 1.1335x over previous
"""GATv2 message passing on 8 Trainium2 NeuronCores (Bass/Tile).

Strategy (edge-parallel by receiver ownership, host-pregathered streams):
  - Nodes split into 8 contiguous ranges of 6250; core c owns range c and all
    edges whose receiver falls in it (no cross-core reduction).
  - The HOST pre-gathers raw endpoint features into receiver-window-sorted
    edge order (pure indexing, same class of prep as the baseline's sort) and
    ships them as bf16 streams laid out feature-major per 128-edge chunk:
    sT[fin, e], rT[fin, e].  The device then does only SEQUENTIAL DMA -- no
    SWDGE gathers at all (the previous bottleneck: 2.4ms of descriptor gen).
  - Per chunk the PE projects both endpoints (lhsT = streamed tile, rhs = Ws
    or Wr in bf16): ps_e = W_s(sent) (the "edges" messages) and ps_x =
    ps_e + W_r(recv) accumulated in PSUM.  mish/logits use the exact
    exp/square/recip chain split across Act/DVE (one act table):
      u = exp(x); w2 = (u+1)^2; w2p1 = w2+1   (Act: exp/square/identity)
      rr = 1/w2p1                             (DVE recip_approx_fast)
      xa2 = x * (2*attn)                      (DVE)
      hm_neg = (rr - 0.5) * xa2 = -mish(x)*attn   (DVE stt)
      lgt_neg = sum_d hm_neg                  (DVE reduce)
      w = exp(-lgt_neg + attn_b)              (Act, scale=-1)
    Messages msg = ps_e * w go into a [e,136] tile whose tail 8 columns hold
    w itself, so ONE one-hot matmul per chunk scatters both the numerator and
    the softmax denominator into the per-window PSUM accumulator.
  - out[n] = segsum(w*msg)/segsum(w), normalized on-device; host reassembles
    the [50000,128] output from the 8 slices.
"""

import os
import sys

for _p in ("/opt/trn_rl_repo", "/root/.axon_site/_ro/trn_rl_repo"):
    if os.path.isdir(_p) and _p not in sys.path:
        sys.path.insert(0, _p)

import numpy as np
import ml_dtypes

import concourse.bass as bass
import concourse.bacc as bacc
import concourse.tile as tile
from concourse import mybir
from concourse import bass_utils

F32 = mybir.dt.float32
BF16 = mybir.dt.bfloat16

N_NODES = 50000
N_EDGES = 800000
F = 128            # feature dim
H = 8              # heads
D = 16             # head dim
NCORE = 8
NPC = N_NODES // NCORE          # 6250 nodes per core
WIN = 128                       # nodes per scatter window
NWIN = (NPC + WIN - 1) // WIN   # 49 windows per core
CHUNK = 128                     # edges per matmul chunk
GRP = 4                         # chunks per elementwise group
BLK = 32                        # chunks per DMA block (4096 edges)
MW = F + H                      # msg+weight columns per chunk (136)

_prog_cache = {}


def _build_program(cpw, nblk, attn_bias, with_xbias, with_wsb):
    """SPMD Bass program: cpw chunks per window, nblk DMA blocks."""
    n_real = NWIN * cpw

    nc = bacc.Bacc("TRN2", target_bir_lowering=False, debug=False,
                   enable_asserts=False, num_devices=NCORE)

    inp = {}
    def dram_in(name, shape, dt=F32):
        inp[name] = nc.dram_tensor(name, list(shape), dt, kind="ExternalInput").ap()
        return inp[name]

    ws_in = dram_in("ws", (F, F), BF16)            # [fin, fout]
    wr_in = dram_in("wr", (F, F), BF16)
    attn2_in = dram_in("attn2", (128, F))          # 2*attn replicated, f32
    iota_in = dram_in("iota", (128, 128), BF16)
    sT_in = dram_in("sT", (nblk, 128, BLK * CHUNK), BF16)
    rT_in = dram_in("rT", (nblk, 128, BLK * CHUNK), BF16)
    rloc_in = dram_in("rloc", (nblk, 128, BLK), BF16)
    if with_xbias:
        xbias_in = dram_in("xbias", (128, F))      # (Ws_b+Wr_b) replicated
    if with_wsb:
        wsb_in = dram_in("wsb", (128, F))          # Ws_b replicated
    out_d = nc.dram_tensor("out_d", [NWIN * WIN, F], F32,
                           kind="ExternalOutput").ap()

    # const AP for the exp bias (activation float biases need registration)
    ab = float(attn_bias)
    if (F32, ab) not in nc.const_aps.aps:
        t = nc.alloc_sbuf_tensor(f"const-ab", [128, 1], F32)
        nc.gpsimd.memset(t.ap(), ab)
        nc.const_aps.aps[(F32, ab)] = t.ap()
        nc.all_engine_barrier()

    def chunk_info(g):
        if g >= n_real:
            return (None, None)
        return divmod(g, cpw)

    with tile.TileContext(nc) as tc:
        with tc.tile_pool(name="const", bufs=1) as cpool, \
             tc.tile_pool(name="stream", bufs=2) as stpool, \
             tc.tile_pool(name="work", bufs=2) as wpool, \
             tc.tile_pool(name="accp", bufs=1) as apool, \
             tc.tile_pool(name="psE", bufs=2, space="PSUM") as psE, \
             tc.tile_pool(name="psR", bufs=2, space="PSUM") as psR, \
             tc.tile_pool(name="psA", bufs=2, space="PSUM") as psA:
            ws_t = cpool.tile([F, F], BF16)
            wr_t = cpool.tile([F, F], BF16)
            attn2_t = cpool.tile([128, F], F32)
            iota_t = cpool.tile([128, 128], BF16)
            nc.sync.dma_start(out=ws_t[:], in_=ws_in[:])
            nc.sync.dma_start(out=wr_t[:], in_=wr_in[:])
            nc.sync.dma_start(out=attn2_t[:], in_=attn2_in[:])
            nc.sync.dma_start(out=iota_t[:], in_=iota_in[:])
            if with_xbias:
                xbias_t = cpool.tile([128, F], F32)
                nc.sync.dma_start(out=xbias_t[:], in_=xbias_in[:])
            if with_wsb:
                wsb_t = cpool.tile([128, F], F32)
                nc.sync.dma_start(out=wsb_t[:], in_=wsb_in[:])

            acc = apool.tile([128, NWIN * MW], F32)

            agg_ps = None
            for b in range(nblk):
                sT_t = stpool.tile([128, BLK * CHUNK], BF16, tag="sT")
                rT_t = stpool.tile([128, BLK * CHUNK], BF16, tag="rT")
                rl_t = stpool.tile([128, BLK], BF16, tag="rl")
                nc.sync.dma_start(out=sT_t[:], in_=sT_in[b])
                nc.sync.dma_start(out=rT_t[:], in_=rT_in[b])
                nc.sync.dma_start(out=rl_t[:], in_=rloc_in[b])

                for g0 in range(0, BLK, GRP):
                    ps_e = psE.tile([128, GRP, F], F32, space="PSUM", tag="pse")
                    ps_x = psR.tile([128, GRP, F], F32, space="PSUM", tag="psx")
                    for c in range(GRP):
                        cc = g0 + c
                        sl = sT_t[:, cc * CHUNK:(cc + 1) * CHUNK]
                        rl = rT_t[:, cc * CHUNK:(cc + 1) * CHUNK]
                        nc.tensor.matmul(ps_e[:, c, :], lhsT=sl, rhs=ws_t[:],
                                         start=True, stop=True,
                                         skip_group_check=True)
                        nc.tensor.matmul(ps_x[:, c, :], lhsT=sl, rhs=ws_t[:],
                                         start=True, stop=False,
                                         skip_group_check=True)
                        nc.tensor.matmul(ps_x[:, c, :], lhsT=rl, rhs=wr_t[:],
                                         start=False, stop=True,
                                         skip_group_check=True)

                    # x = s_proj + r_proj accumulated on PE; optional bias add
                    if with_xbias:
                        x_sb = wpool.tile([128, GRP, F], F32, tag="x")
                        nc.vector.tensor_tensor(
                            x_sb[:], ps_x[:],
                            xbias_t[:].unsqueeze(1).to_broadcast([128, GRP, F]),
                            op=mybir.AluOpType.add)
                        x = x_sb[:]
                    else:
                        x = ps_x[:]
                    # xa2 = x * 2*attn              [DVE: reads PSUM]
                    xa2 = wpool.tile([128, GRP, F], F32, tag="xa2")
                    nc.vector.tensor_tensor(
                        xa2[:], x,
                        attn2_t[:].unsqueeze(1).to_broadcast([128, GRP, F]),
                        op=mybir.AluOpType.mult)
                    # one-hot rows                  [DVE]
                    oh = wpool.tile([128, GRP, 128], BF16, tag="oh")
                    nc.vector.tensor_tensor(
                        oh[:],
                        rl_t[:, g0:g0 + GRP].unsqueeze(2)
                            .to_broadcast([128, GRP, 128]),
                        iota_t[:].unsqueeze(1).to_broadcast([128, GRP, 128]),
                        op=mybir.AluOpType.is_equal)

                    # mish chain                    [Act]
                    u = wpool.tile([128, GRP, F], F32, tag="u")
                    nc.scalar.activation(u[:], x,
                                         mybir.ActivationFunctionType.Exp)
                    w2 = wpool.tile([128, GRP, F], F32, tag="w2")
                    nc.scalar.activation(w2[:], u[:],
                                         mybir.ActivationFunctionType.Square,
                                         bias=1.0)
                    w2p1 = wpool.tile([128, GRP, F], F32, tag="w2p1")
                    nc.scalar.activation(w2p1[:], w2[:],
                                         mybir.ActivationFunctionType.Identity,
                                         bias=1.0)

                    # rr = 1/((u+1)^2+1)            [DVE]
                    rr = wpool.tile([128, GRP, F], F32, tag="rr")
                    nc.vector.reciprocal_approx_fast(rr[:], w2p1[:])
                    # hm_neg = (rr-0.5)*xa2 = -mish(x)*attn  [DVE]
                    hm = wpool.tile([128, GRP, F], BF16, tag="hm")
                    nc.vector.scalar_tensor_tensor(
                        hm[:], rr[:], 0.5, xa2[:],
                        op0=mybir.AluOpType.subtract,
                        op1=mybir.AluOpType.mult)
                    # lgt_neg = sum_d hm_neg        [DVE]
                    lgt = wpool.tile([128, GRP, H], F32, tag="lgt")
                    nc.vector.tensor_reduce(
                        out=lgt[:].unsqueeze(3),
                        in_=hm[:].rearrange("p c (h d) -> p c h d", d=D),
                        op=mybir.AluOpType.add, axis=mybir.AxisListType.X)

                    # w = exp(-lgt_neg + attn_b)    [Act]
                    wv = wpool.tile([128, GRP, H], F32, tag="wv")
                    nc.scalar.activation(wv[:], lgt[:],
                                         mybir.ActivationFunctionType.Exp,
                                         bias=ab, scale=-1.0)
                    msgw = wpool.tile([128, GRP, MW], BF16, tag="msgw")
                    nc.scalar.activation(msgw[:, :, F:MW], lgt[:],
                                         mybir.ActivationFunctionType.Exp,
                                         bias=ab, scale=-1.0)
                    # msg = s_proj * w              [DVE]
                    nc.vector.tensor_tensor(
                        msgw[:, :, 0:F].rearrange("p c (h d) -> p c h d", d=D),
                        ps_e[:].rearrange("p c (h d) -> p c h d", d=D),
                        wv[:].unsqueeze(3).to_broadcast([128, GRP, H, D]),
                        op=mybir.AluOpType.mult)

                    # scatter                       [PE]
                    for c in range(GRP):
                        g_ch = b * BLK + g0 + c
                        w_idx, pos = chunk_info(g_ch)
                        if w_idx is None:
                            continue
                        if pos == 0:
                            agg_ps = psA.tile([128, MW], F32, space="PSUM",
                                              tag="agg")
                        nc.tensor.matmul(agg_ps[:], lhsT=oh[:, c, :],
                                         rhs=msgw[:, c, :],
                                         start=(pos == 0),
                                         stop=(pos == cpw - 1),
                                         skip_group_check=True)
                        if pos == cpw - 1:
                            nc.scalar.copy(acc[:, w_idx * MW:(w_idx + 1) * MW],
                                           agg_ps[:])

            # ---------------- normalize + store ----------------
            accv = acc[:].rearrange("p (w k) -> p w k", k=MW)
            den = accv[:, :, F:MW]
            nc.vector.tensor_scalar_add(den, den, 1e-30)
            rcp = wpool.tile([128, NWIN * H], F32, tag="rcp")
            scr = wpool.tile([128, NWIN * H], F32, tag="scr")
            nc.vector.reciprocal_approx_accurate(
                rcp[:].rearrange("p (w h) -> p w h", h=H), den, scr[:])
            outb = wpool.tile([128, NWIN * F], F32, tag="outb")
            nc.vector.tensor_tensor(
                outb[:].rearrange("p (w h d) -> p w h d", h=H, d=D),
                accv[:, :, 0:F].rearrange("p w (h d) -> p w h d", d=D),
                rcp[:].rearrange("p (w h) -> p w h", h=H).unsqueeze(3)
                      .to_broadcast([128, NWIN, H, D]),
                op=mybir.AluOpType.mult)
            if with_wsb:
                nc.vector.tensor_tensor(
                    outb[:].rearrange("p (w f) -> p w f", f=F),
                    outb[:].rearrange("p (w f) -> p w f", f=F),
                    wsb_t[:].unsqueeze(1).to_broadcast([128, NWIN, F]),
                    op=mybir.AluOpType.add)
            nc.sync.dma_start(
                out=out_d[:].rearrange("(w p) f -> p w f", p=128),
                in_=outb[:].rearrange("p (w f) -> p w f", f=F))

    nc.compile()
    return nc


def _prep_core(nodes_bf, senders, receivers, core, cpw, nblk):
    """Pre-gather the per-core edge streams (host-side indexing only)."""
    e_pad = nblk * BLK * CHUNK
    mask = (receivers // NPC) == core
    s = senders[mask].astype(np.int64)
    r = receivers[mask].astype(np.int64)
    rl = r - core * NPC
    win = rl // WIN
    order = np.argsort(win, kind="stable")
    s, r, rl, win = s[order], r[order], rl[order], win[order]

    # slot of each edge: window base + rank within window
    pos = np.arange(len(win)) - np.searchsorted(win, win)
    slot = win * (cpw * CHUNK) + pos
    assert pos.max(initial=0) < cpw * CHUNK

    sidx = np.zeros(e_pad, np.int64)
    ridx = np.zeros(e_pad, np.int64)
    rloc_val = np.full(e_pad, 999.0, np.float32)
    sidx[slot] = s
    ridx[slot] = r
    rloc_val[slot] = (rl - win * WIN).astype(np.float32)

    # feature-major bf16 streams: [nblk, 128 fin, BLK*CHUNK edges]
    sT = nodes_bf[sidx].reshape(nblk, BLK * CHUNK, F).transpose(0, 2, 1).copy()
    rT = nodes_bf[ridx].reshape(nblk, BLK * CHUNK, F).transpose(0, 2, 1).copy()
    rloc = rloc_val.reshape(nblk, BLK, CHUNK).transpose(0, 2, 1).astype(
        ml_dtypes.bfloat16).copy()
    return sT, rT, rloc


def kernel(nodes, senders, receivers, Ws_k, Ws_b, Wr_k, Wr_b, attn_k, attn_b):
    nodes = np.asarray(nodes, np.float32)
    senders = np.asarray(senders, np.int32)
    receivers = np.asarray(receivers, np.int32)
    assert nodes.shape == (N_NODES, F) and senders.shape == (N_EDGES,)

    core_of = receivers // NPC
    win = (receivers - core_of * NPC) // WIN
    key = core_of.astype(np.int64) * NWIN + win
    counts = np.bincount(key, minlength=NCORE * NWIN)
    cpw = max(1, int(np.ceil(counts.max() / CHUNK)))
    nblk = (NWIN * cpw + BLK - 1) // BLK

    wsb = np.asarray(Ws_b, np.float32).reshape(F)
    wrb = np.asarray(Wr_b, np.float32).reshape(F)
    ab = float(np.asarray(attn_b, np.float32).ravel()[0])
    with_xbias = bool(np.any(wsb != 0) or np.any(wrb != 0))
    with_wsb = bool(np.any(wsb != 0))

    ck = (cpw, nblk, ab, with_xbias, with_wsb)
    if ck not in _prog_cache:
        _prog_cache[ck] = _build_program(*ck)
    nc = _prog_cache[ck]

    nodes_bf = nodes.astype(ml_dtypes.bfloat16)
    ws = np.asarray(Ws_k, np.float32).reshape(F, F).astype(ml_dtypes.bfloat16)
    wr = np.asarray(Wr_k, np.float32).reshape(F, F).astype(ml_dtypes.bfloat16)
    a_flat = np.tile(np.asarray(attn_k, np.float32).ravel(), H)
    attn2 = np.broadcast_to(2.0 * a_flat, (128, F)).copy()
    iota = np.broadcast_to(np.arange(128, dtype=np.float32),
                           (128, 128)).astype(ml_dtypes.bfloat16).copy()

    in_maps = []
    for c in range(NCORE):
        sT, rT, rloc = _prep_core(nodes_bf, senders, receivers, c, cpw, nblk)
        im = {"ws": ws, "wr": wr, "attn2": attn2, "iota": iota,
              "sT": sT, "rT": rT, "rloc": rloc}
        if with_xbias:
            im["xbias"] = np.broadcast_to(wsb + wrb, (128, F)).copy()
        if with_wsb:
            im["wsb"] = np.broadcast_to(wsb, (128, F)).copy()
        in_maps.append(im)

    trace = bool(int(os.environ.get("GAT_TRACE", "0")))
    res = bass_utils.run_bass_kernel_spmd(nc, in_maps,
                                          core_ids=list(range(NCORE)),
                                          trace=trace)
    if trace:
        kernel.last_profile = res
    out = np.empty((N_NODES, F), np.float32)
    for c in range(NCORE):
        out[c * NPC:(c + 1) * NPC] = np.asarray(res.results[c]["out_d"])[:NPC]
    return out


# revision 19
# speedup vs baseline: 1.3927x; 1.0050x over previous
"""GATv2 message passing on 8 Trainium2 NeuronCores (Bass/Tile).

Strategy (edge-parallel by receiver ownership, host-pregathered streams):
  - Nodes split into 8 contiguous ranges of 6250; core c owns range c and all
    edges whose receiver falls in it (no cross-core reduction).
  - The HOST pre-gathers raw endpoint features into receiver-window-sorted
    edge order (pure indexing, same class of prep as the baseline's sort) and
    ships them as bf16 streams laid out feature-major per 128-edge chunk:
    sT[fin, e], rT[fin, e].  The device then does only SEQUENTIAL DMA -- no
    SWDGE gathers at all (the previous bottleneck: 2.4ms of descriptor gen).
  - Per chunk the PE projects both endpoints (lhsT = streamed tile, rhs = Ws
    or Wr in bf16): ps_e = W_s(sent) (the "edges" messages) and ps_x =
    ps_e + W_r(recv) accumulated in PSUM.  mish/logits use the exact
    exp/square/recip chain split across Act/DVE (one act table):
      u = exp(x); w2 = (u+1)^2; w2p1 = w2+1   (Act: exp/square/identity)
      rr = 1/w2p1                             (DVE recip_approx_fast)
      xa2 = x * (2*attn)                      (DVE)
      hm_neg = (rr - 0.5) * xa2 = -mish(x)*attn   (DVE stt)
      lgt_neg = sum_d hm_neg                  (DVE reduce)
      w = exp(-lgt_neg + attn_b)              (Act, scale=-1)
    Messages msg = ps_e * w go into a [e,136] tile whose tail 8 columns hold
    w itself, so ONE one-hot matmul per chunk scatters both the numerator and
    the softmax denominator into the per-window PSUM accumulator.
  - out[n] = segsum(w*msg)/segsum(w), normalized on-device; host reassembles
    the [50000,128] output from the 8 slices.
"""

import os
import sys

for _p in ("/opt/trn_rl_repo", "/root/.axon_site/_ro/trn_rl_repo"):
    if os.path.isdir(_p) and _p not in sys.path:
        sys.path.insert(0, _p)

import numpy as np
import ml_dtypes

import concourse.bass as bass
import concourse.bacc as bacc
import concourse.tile as tile
from concourse import mybir
from concourse import bass_utils

F32 = mybir.dt.float32
BF16 = mybir.dt.bfloat16

N_NODES = 50000
N_EDGES = 800000
F = 128            # feature dim
H = 8              # heads
D = 16             # head dim
NCORE = 8
NPC = N_NODES // NCORE          # 6250 nodes per core
WIN = 128                       # nodes per scatter window
NWIN = (NPC + WIN - 1) // WIN   # 49 windows per core
CHUNK = 128                     # edges per matmul chunk
GRP = 4                         # chunks per elementwise group
BLK = 32                        # chunks per DMA block (4096 edges)
MW = F + H                      # msg+weight columns per chunk (136)

_prog_cache = {}


def _build_program(cpw, nblk, attn_bias, with_xbias, with_wsb):
    """SPMD Bass program: cpw chunks per window, nblk DMA blocks."""
    n_real = NWIN * cpw

    nc = bacc.Bacc("TRN2", target_bir_lowering=False, debug=False,
                   enable_asserts=False, num_devices=NCORE)

    inp = {}
    def dram_in(name, shape, dt=F32):
        inp[name] = nc.dram_tensor(name, list(shape), dt, kind="ExternalInput").ap()
        return inp[name]

    ws_in = dram_in("ws", (F, F), BF16)            # [fin, fout]
    wr_in = dram_in("wr", (F, F), BF16)
    attn2_in = dram_in("attn2", (128, F))          # 2*attn replicated, f32
    iota_in = dram_in("iota", (128, 128), BF16)
    sT_in = dram_in("sT", (nblk, 128, BLK * CHUNK), BF16)
    rT_in = dram_in("rT", (nblk, 128, BLK * CHUNK), BF16)
    rloc_in = dram_in("rloc", (nblk, 128, BLK), BF16)
    if with_xbias:
        xbias_in = dram_in("xbias", (128, F))      # (Ws_b+Wr_b) replicated
    if with_wsb:
        wsb_in = dram_in("wsb", (128, F))          # Ws_b replicated
    out_d = nc.dram_tensor("out_d", [NWIN * WIN, F], F32,
                           kind="ExternalOutput").ap()

    # const AP for the exp bias (activation float biases need registration)
    ab = float(attn_bias)
    if (F32, ab) not in nc.const_aps.aps:
        t = nc.alloc_sbuf_tensor(f"const-ab", [128, 1], F32)
        nc.gpsimd.memset(t.ap(), ab)
        nc.const_aps.aps[(F32, ab)] = t.ap()
        nc.all_engine_barrier()

    def chunk_info(g):
        if g >= n_real:
            return (None, None)
        return divmod(g, cpw)

    with tile.TileContext(nc) as tc:
        with tc.tile_pool(name="const", bufs=1) as cpool, \
             tc.tile_pool(name="stream", bufs=2) as stpool, \
             tc.tile_pool(name="work", bufs=2) as wpool, \
             tc.tile_pool(name="accp", bufs=1) as apool, \
             tc.tile_pool(name="psE", bufs=2, space="PSUM") as psE, \
             tc.tile_pool(name="psR", bufs=2, space="PSUM") as psR, \
             tc.tile_pool(name="psA", bufs=2, space="PSUM") as psA:
            ws_t = cpool.tile([F, F], BF16)
            wr_t = cpool.tile([F, F], BF16)
            attn2_t = cpool.tile([128, F], F32)
            iota_t = cpool.tile([128, 128], BF16)
            nc.sync.dma_start(out=ws_t[:], in_=ws_in[:])
            nc.sync.dma_start(out=wr_t[:], in_=wr_in[:])
            nc.sync.dma_start(out=attn2_t[:], in_=attn2_in[:])
            nc.sync.dma_start(out=iota_t[:], in_=iota_in[:])
            if with_xbias:
                xbias_t = cpool.tile([128, F], F32)
                nc.sync.dma_start(out=xbias_t[:], in_=xbias_in[:])
            if with_wsb:
                wsb_t = cpool.tile([128, F], F32)
                nc.sync.dma_start(out=wsb_t[:], in_=wsb_in[:])

            acc = apool.tile([128, NWIN * MW], F32)

            agg_ps = None
            for b in range(nblk):
                sT_t = stpool.tile([128, BLK * CHUNK], BF16, tag="sT")
                rT_t = stpool.tile([128, BLK * CHUNK], BF16, tag="rT")
                rl_t = stpool.tile([128, BLK], BF16, tag="rl")
                nc.sync.dma_start(out=sT_t[:], in_=sT_in[b])
                nc.sync.dma_start(out=rT_t[:], in_=rT_in[b])
                nc.sync.dma_start(out=rl_t[:], in_=rloc_in[b])

                for g0 in range(0, BLK, GRP):
                    if b * BLK + g0 >= n_real:
                        continue   # fully-padded group: scatter never reads it
                    ps_e = psE.tile([128, GRP, F], F32, space="PSUM", tag="pse")
                    ps_x = psR.tile([128, GRP, F], F32, space="PSUM", tag="psx")
                    for c in range(GRP):
                        cc = g0 + c
                        sl = sT_t[:, cc * CHUNK:(cc + 1) * CHUNK]
                        rl = rT_t[:, cc * CHUNK:(cc + 1) * CHUNK]
                        nc.tensor.matmul(ps_e[:, c, :], lhsT=sl, rhs=ws_t[:],
                                         start=True, stop=True,
                                         skip_group_check=True)
                        nc.tensor.matmul(ps_x[:, c, :], lhsT=sl, rhs=ws_t[:],
                                         start=True, stop=False,
                                         skip_group_check=True)
                        nc.tensor.matmul(ps_x[:, c, :], lhsT=rl, rhs=wr_t[:],
                                         start=False, stop=True,
                                         skip_group_check=True)

                    # x = s_proj + r_proj accumulated on PE; optional bias add
                    if with_xbias:
                        x_sb = wpool.tile([128, GRP, F], F32, tag="x")
                        nc.vector.tensor_tensor(
                            x_sb[:], ps_x[:],
                            xbias_t[:].unsqueeze(1).to_broadcast([128, GRP, F]),
                            op=mybir.AluOpType.add)
                        x = x_sb[:]
                    else:
                        x = ps_x[:]
                    # xa2 = x * 2*attn              [DVE: reads PSUM]
                    xa2 = wpool.tile([128, GRP, F], F32, tag="xa2")
                    nc.vector.tensor_tensor(
                        xa2[:], x,
                        attn2_t[:].unsqueeze(1).to_broadcast([128, GRP, F]),
                        op=mybir.AluOpType.mult)
                    # one-hot rows                  [DVE]
                    oh = wpool.tile([128, GRP, 128], BF16, tag="oh")
                    nc.vector.tensor_tensor(
                        oh[:],
                        rl_t[:, g0:g0 + GRP].unsqueeze(2)
                            .to_broadcast([128, GRP, 128]),
                        iota_t[:].unsqueeze(1).to_broadcast([128, GRP, 128]),
                        op=mybir.AluOpType.is_equal)

                    # mish chain                    [Act]
                    u = wpool.tile([128, GRP, F], F32, tag="u")
                    nc.scalar.activation(u[:], x,
                                         mybir.ActivationFunctionType.Exp)
                    w2 = wpool.tile([128, GRP, F], F32, tag="w2")
                    nc.scalar.activation(w2[:], u[:],
                                         mybir.ActivationFunctionType.Square,
                                         bias=1.0)
                    w2p1 = wpool.tile([128, GRP, F], F32, tag="w2p1")
                    nc.scalar.activation(w2p1[:], w2[:],
                                         mybir.ActivationFunctionType.Identity,
                                         bias=1.0)

                    # rr = 1/((u+1)^2+1)            [DVE]
                    rr = wpool.tile([128, GRP, F], F32, tag="rr")
                    nc.vector.reciprocal_approx_fast(rr[:], w2p1[:])
                    # hm_neg = (rr-0.5)*xa2 = -mish(x)*attn  [DVE]
                    hm = wpool.tile([128, GRP, F], BF16, tag="hm")
                    nc.vector.scalar_tensor_tensor(
                        hm[:], rr[:], 0.5, xa2[:],
                        op0=mybir.AluOpType.subtract,
                        op1=mybir.AluOpType.mult)
                    # lgt_neg = sum_d hm_neg        [DVE]
                    lgt = wpool.tile([128, GRP, H], F32, tag="lgt")
                    nc.vector.tensor_reduce(
                        out=lgt[:].unsqueeze(3),
                        in_=hm[:].rearrange("p c (h d) -> p c h d", d=D),
                        op=mybir.AluOpType.add, axis=mybir.AxisListType.X)

                    # w = exp(-lgt_neg + attn_b)    [Act]
                    wv = wpool.tile([128, GRP, H], F32, tag="wv")
                    nc.scalar.activation(wv[:], lgt[:],
                                         mybir.ActivationFunctionType.Exp,
                                         bias=ab, scale=-1.0)
                    msgw = wpool.tile([128, GRP, MW], BF16, tag="msgw")
                    nc.scalar.activation(msgw[:, :, F:MW], lgt[:],
                                         mybir.ActivationFunctionType.Exp,
                                         bias=ab, scale=-1.0)
                    # msg = s_proj * w              [DVE]
                    nc.vector.tensor_tensor(
                        msgw[:, :, 0:F].rearrange("p c (h d) -> p c h d", d=D),
                        ps_e[:].rearrange("p c (h d) -> p c h d", d=D),
                        wv[:].unsqueeze(3).to_broadcast([128, GRP, H, D]),
                        op=mybir.AluOpType.mult)

                    # scatter                       [PE]
                    for c in range(GRP):
                        g_ch = b * BLK + g0 + c
                        w_idx, pos = chunk_info(g_ch)
                        if w_idx is None:
                            continue
                        if pos == 0:
                            agg_ps = psA.tile([128, MW], F32, space="PSUM",
                                              tag="agg")
                        nc.tensor.matmul(agg_ps[:], lhsT=oh[:, c, :],
                                         rhs=msgw[:, c, :],
                                         start=(pos == 0),
                                         stop=(pos == cpw - 1),
                                         skip_group_check=True)
                        if pos == cpw - 1:
                            nc.scalar.copy(acc[:, w_idx * MW:(w_idx + 1) * MW],
                                           agg_ps[:])

            # ---------------- normalize + store ----------------
            accv = acc[:].rearrange("p (w k) -> p w k", k=MW)
            den = accv[:, :, F:MW]
            nc.vector.tensor_scalar_add(den, den, 1e-30)
            rcp = wpool.tile([128, NWIN * H], F32, tag="rcp")
            scr = wpool.tile([128, NWIN * H], F32, tag="scr")
            nc.vector.reciprocal_approx_accurate(
                rcp[:].rearrange("p (w h) -> p w h", h=H), den, scr[:])
            outb = wpool.tile([128, NWIN * F], F32, tag="outb")
            nc.vector.tensor_tensor(
                outb[:].rearrange("p (w h d) -> p w h d", h=H, d=D),
                accv[:, :, 0:F].rearrange("p w (h d) -> p w h d", d=D),
                rcp[:].rearrange("p (w h) -> p w h", h=H).unsqueeze(3)
                      .to_broadcast([128, NWIN, H, D]),
                op=mybir.AluOpType.mult)
            if with_wsb:
                nc.vector.tensor_tensor(
                    outb[:].rearrange("p (w f) -> p w f", f=F),
                    outb[:].rearrange("p (w f) -> p w f", f=F),
                    wsb_t[:].unsqueeze(1).to_broadcast([128, NWIN, F]),
                    op=mybir.AluOpType.add)
            nc.sync.dma_start(
                out=out_d[:].rearrange("(w p) f -> p w f", p=128),
                in_=outb[:].rearrange("p (w f) -> p w f", f=F))

    nc.compile()
    return nc


def _prep_core(nodes_bf, senders, receivers, core, cpw, nblk):
    """Pre-gather the per-core edge streams (host-side indexing only)."""
    e_pad = nblk * BLK * CHUNK
    mask = (receivers // NPC) == core
    s = senders[mask].astype(np.int64)
    r = receivers[mask].astype(np.int64)
    rl = r - core * NPC
    win = rl // WIN
    order = np.argsort(win, kind="stable")
    s, r, rl, win = s[order], r[order], rl[order], win[order]

    # slot of each edge: window base + rank within window
    pos = np.arange(len(win)) - np.searchsorted(win, win)
    slot = win * (cpw * CHUNK) + pos
    assert pos.max(initial=0) < cpw * CHUNK

    sidx = np.zeros(e_pad, np.int64)
    ridx = np.zeros(e_pad, np.int64)
    rloc_val = np.full(e_pad, 999.0, np.float32)
    sidx[slot] = s
    ridx[slot] = r
    rloc_val[slot] = (rl - win * WIN).astype(np.float32)

    # feature-major bf16 streams: [nblk, 128 fin, BLK*CHUNK edges]
    sT = nodes_bf[sidx].reshape(nblk, BLK * CHUNK, F).transpose(0, 2, 1).copy()
    rT = nodes_bf[ridx].reshape(nblk, BLK * CHUNK, F).transpose(0, 2, 1).copy()
    rloc = rloc_val.reshape(nblk, BLK, CHUNK).transpose(0, 2, 1).astype(
        ml_dtypes.bfloat16).copy()
    return sT, rT, rloc


def kernel(nodes, senders, receivers, Ws_k, Ws_b, Wr_k, Wr_b, attn_k, attn_b):
    nodes = np.asarray(nodes, np.float32)
    senders = np.asarray(senders, np.int32)
    receivers = np.asarray(receivers, np.int32)
    assert nodes.shape == (N_NODES, F) and senders.shape == (N_EDGES,)

    core_of = receivers // NPC
    win = (receivers - core_of * NPC) // WIN
    key = core_of.astype(np.int64) * NWIN + win
    counts = np.bincount(key, minlength=NCORE * NWIN)
    cpw = max(1, int(np.ceil(counts.max() / CHUNK)))
    nblk = (NWIN * cpw + BLK - 1) // BLK

    wsb = np.asarray(Ws_b, np.float32).reshape(F)
    wrb = np.asarray(Wr_b, np.float32).reshape(F)
    ab = float(np.asarray(attn_b, np.float32).ravel()[0])
    with_xbias = bool(np.any(wsb != 0) or np.any(wrb != 0))
    with_wsb = bool(np.any(wsb != 0))

    ck = (cpw, nblk, ab, with_xbias, with_wsb)
    if ck not in _prog_cache:
        _prog_cache[ck] = _build_program(*ck)
    nc = _prog_cache[ck]

    nodes_bf = nodes.astype(ml_dtypes.bfloat16)
    ws = np.asarray(Ws_k, np.float32).reshape(F, F).astype(ml_dtypes.bfloat16)
    wr = np.asarray(Wr_k, np.float32).reshape(F, F).astype(ml_dtypes.bfloat16)
    a_flat = np.tile(np.asarray(attn_k, np.float32).ravel(), H)
    attn2 = np.broadcast_to(2.0 * a_flat, (128, F)).copy()
    iota = np.broadcast_to(np.arange(128, dtype=np.float32),
                           (128, 128)).astype(ml_dtypes.bfloat16).copy()

    in_maps = []
    for c in range(NCORE):
        sT, rT, rloc = _prep_core(nodes_bf, senders, receivers, c, cpw, nblk)
        im = {"ws": ws, "wr": wr, "attn2": attn2, "iota": iota,
              "sT": sT, "rT": rT, "rloc": rloc}
        if with_xbias:
            im["xbias"] = np.broadcast_to(wsb + wrb, (128, F)).copy()
        if with_wsb:
            im["wsb"] = np.broadcast_to(wsb, (128, F)).copy()
        in_maps.append(im)

    trace = bool(int(os.environ.get("GAT_TRACE", "0")))
    res = bass_utils.run_bass_kernel_spmd(nc, in_maps,
                                          core_ids=list(range(NCORE)),
                                          trace=trace)
    if trace:
        kernel.last_profile = res
    out = np.empty((N_NODES, F), np.float32)
    for c in range(NCORE):
        out[c * NPC:(c + 1) * NPC] = np.asarray(res.results[c]["out_d"])[:NPC]
    return out
